# revision 48
# baseline (speedup 1.0000x reference)
"""Trainium2 Bass kernel for nn_LiquidModel (moe_routing) — v5.

Host does all algebraic folding; device does 12 GEMM-units + attention:
 - Degenerate MoE routing (experts picked by token 0, averaged) -> the three
   MoE layers fold into ONE dense GEMM (W1@W2@W3, f64 on host).  Same for
   ffw@cfw and k2w@outw in the trailing stack.
 - Dense GEMMs run bf16 (fp32 PSUM accumulate); residual stream is bf16.
 - q/k/v GEMMs run fp8 DoubleRow (256-deep contraction per instruction);
   their outputs are fp8-quantized for the exchange anyway.
 - K/V exchange: 4 AllGathers, one per HEAD (k head-slice [256,512] + v
   head-slice [512,256], fp8, 256KB payload).  Attention chunk h consumes
   AllGather h; the head's output accumulates fully in PSUM (no SBUF
   folds) and its epilogue runs inline.
 - Softmax uniform part is EXACT: sum_t v comes from the host column-sum
   identity, so fp8 noise only touches the small es*v correction term.
 - LN1/LN2 are folded into the following GEMM (diag(g)@W on host + rank-1
   mu*rstd correction matmul + per-token rstd scale at eviction), so the
   tensor engine never waits for LN row math.
"""
import ml_dtypes
import numpy as np

import concourse.bacc as bacc
import concourse.bass as bass
import concourse.mybir as mybir
import concourse.tile as tile
from concourse import bass_utils

FP32 = mybir.dt.float32
FP32R = mybir.dt.float32r
BF16 = mybir.dt.bfloat16
FP8 = mybir.dt.float8e4
AF = mybir.ActivationFunctionType
ALU = mybir.AluOpType
DR = mybir.MatmulPerfMode.DoubleRow

NCORES = 8
N, D, DFF, H, L = 4096, 1024, 2048, 4, 3
TOK = N // NCORES          # 512 tokens per core
DH = D // H                # 256
EPS = 1e-5
KC = D // 128              # 8 feature chunks of 128
P = 128

_CACHE = {}


def _body(nc, tc, io):
    # ---- persistent SBUF activation tensors (feature-major [128, TOK]) ----
    xin = [nc.alloc_sbuf_tensor(f"xin{i}", [P, TOK], BF16).ap() for i in range(KC)]
    x3b = [nc.alloc_sbuf_tensor(f"x3b{i}", [P, TOK], BF16).ap() for i in range(KC)]
    x8 = [nc.alloc_sbuf_tensor(f"x8_{i}", [P, 2 * TOK], FP8).ap() for i in range(KC // 2)]
    xRb = [nc.alloc_sbuf_tensor(f"xRb{i}", [P, TOK], BF16).ap() for i in range(KC)]
    yA = [nc.alloc_sbuf_tensor(f"yA{i}", [P, TOK], BF16).ap() for i in range(KC)]
    yB = [nc.alloc_sbuf_tensor(f"yB{i}", [P, TOK], BF16).ap() for i in range(KC)]
    hT = [nc.alloc_sbuf_tensor(f"hT{i}", [P, TOK], BF16).ap() for i in range(2 * KC)]
    q8 = [nc.alloc_sbuf_tensor(f"q8_{h}", [P, 2 * TOK], FP8).ap() for h in range(H)]
    oT = xin      # attention output reuses the input slots (dead after MoE)
    xR2b = xin    # and the second residual reuses them again after o-proj

    with (
        tc.tile_pool(name="const", bufs=1) as cp,
        tc.tile_pool(name="wp", bufs=10) as wp,
        tc.tile_pool(name="sp", bufs=4) as sp,
        tc.tile_pool(name="dram", bufs=1, space="DRAM") as dp,
    ):
        # ---- input x loads first (gpsimd queue, feeds the MoE GEMM asap) ----
        for i in range(KC):
            nc.gpsimd.dma_start(xin[i][:], io["xT"][i * P:(i + 1) * P, :])

        # ---- constants (gpsimd queue) ----
        onesb_col = cp.tile([P, 1], BF16, tag="onesb_col")
        nc.gpsimd.dma_start(onesb_col[:], io["c_onesb"][0:128].rearrange("(p o) -> p o", o=1))
        onesb_row = cp.tile([1, P], BF16, tag="onesb_row")
        nc.gpsimd.dma_start(onesb_row[:], io["c_onesb"][0:128].rearrange("(o p) -> o p", o=1))
        ones_row = cp.tile([1, P], FP32R, tag="ones_row")
        nc.gpsimd.dma_start(ones_row[:], io["c_ones"][0:128].rearrange("(o p) -> o p", o=1))
        eye = cp.tile([P, P], FP32R, tag="eye")
        nc.gpsimd.dma_start(eye[:], io["c_eye"][:, :])
        eps_t = cp.tile([1, 1], FP32, tag="eps")
        nc.vector.memset(eps_t[:], EPS)
        vb_row = cp.tile([1, D], BF16, tag="vb_row")
        nc.gpsimd.dma_start(vb_row[:], io["vb2048"][:].rearrange("(o d) -> o d", o=1))
        vrow_r = cp.tile([1, H * (DH + 2)], FP32R, tag="vrow")
        nc.gpsimd.dma_start(vrow_r[:], io["vrow"][:].rearrange("(o d) -> o d", o=1))
        c1f1_r = cp.tile([1, DFF], FP32R, tag="c1f1")
        nc.gpsimd.dma_start(c1f1_r[:], io["c1f1"][:].rearrange("(o d) -> o d", o=1))
        c1fc_r = cp.tile([1, D], FP32R, tag="c1fc")
        nc.gpsimd.dma_start(c1fc_r[:], io["c1fc"][:].rearrange("(o d) -> o d", o=1))

        def vec_tile(name, length):
            cols = length // P
            t = cp.tile([P, cols], FP32, tag=f"vec_{name}")
            nc.gpsimd.dma_start(t[:], io[name][:].rearrange("(c p) -> p c", p=P))
            return t

        moeb_t = vec_tile("moeb8", D)       # 8*moeb (for the fp8 x8 evict)
        moebf_t = vec_tile("moeb", D)       # moeb   (for the bf16 x3 evict)
        qkb16_t = vec_tile("qkb16", 2 * D)
        qkb2048_t = vec_tile("qkb2048", 2 * D)
        ob_t = vec_tile("ob", D)
        f1b_t = vec_tile("f1b", DFF)        # ln1b@f1w + f1b (host-folded)
        f2b_t = vec_tile("f2b", D)
        ln1g_t = vec_tile("ln1g", D)
        ln1b_t = vec_tile("ln1b", D)
        fcb_t = vec_tile("fcb", D)          # ln2b@fcw + bfc (host-folded)
        k1b_t = vec_tile("k1b", D)
        kob_t = vec_tile("kob", D)

        # ---- DRAM buffers: one exchange chunk per HEAD ----
        # kv_loc[h] = [1024, 256] fp8: rows 0..511 = k head-slice (row =
        # s*256 + p*2 + a: feature s*128+p, token-half a); rows 512..1023 =
        # v head-slice, one row per token.
        kv_loc = [dp.tile([1024, 256], FP8, tag=f"kv_loc{h}", name=f"kv_loc{h}")
                  for h in range(H)]
        kv_all = [dp.tile([NCORES * 1024, 256], FP8, tag=f"kv_all{h}",
                          name=f"kv_all{h}", addr_space="Shared")
                  for h in range(H)]

        # ------------------------------------------------------------------
        # dense feature-major GEMM:  psum[M-chunk, TOK] = W[K, M]^T-contr x^T
        # evict(m, pss) per output chunk m.  rank1=(c1_row, r1_getter) adds
        # the LN-fold rank-1 correction before closing the accumulation.
        # ------------------------------------------------------------------
        def gemm_fm(w_ap, K, M, x_tiles, evict, psum_pool, engines=None,
                    rank1=None, m_outer=False):
            engines = engines or [nc.sync, nc.scalar]
            kc = K // P
            for half in range(M // 1024):
                if m_outer:
                    # resident weights; each m-chunk closes early so the
                    # eviction chain pipelines under the remaining matmuls
                    wts = []
                    for k in range(kc):
                        wt = wp.tile([P, 1024], BF16, tag="w", bufs=12)
                        engines[k % len(engines)].dma_start(
                            wt[:], w_ap[k * P:(k + 1) * P,
                                        half * 1024:(half + 1) * 1024])
                        wts.append(wt)
                    for m2 in range(8):
                        ps = psum_pool.tile([P, TOK], FP32, tag="mm", bufs=8,
                                            name=f"psmo{half}_{m2}")
                        for k in range(kc):
                            nc.tensor.matmul(
                                ps[:], wts[k][:, m2 * P:(m2 + 1) * P],
                                x_tiles[k][:], start=(k == 0),
                                stop=(k == kc - 1 and rank1 is None))
                        if rank1 is not None:
                            c1_row, r1_row = rank1
                            nc.tensor.matmul(
                                ps[:],
                                c1_row[0:1, half * 1024 + m2 * P:
                                       half * 1024 + (m2 + 1) * P],
                                r1_row[:], start=False, stop=True,
                                skip_group_check=True)
                        evict(half * 8 + m2, ps)
                    continue
                pss = [psum_pool.tile([P, TOK], FP32, tag="mm", bufs=8,
                                      name=f"ps{half}_{i}") for i in range(8)]
                for k in range(kc):
                    wt = wp.tile([P, 1024], BF16, tag="w", bufs=12)
                    engines[k % len(engines)].dma_start(
                        wt[:],
                        w_ap[k * P:(k + 1) * P,
                             half * 1024:(half + 1) * 1024])
                    for m2 in range(8):
                        nc.tensor.matmul(
                            pss[m2][:], wt[:, m2 * P:(m2 + 1) * P],
                            x_tiles[k][:],
                            start=(k == 0),
                            stop=(k == kc - 1 and rank1 is None))
                if rank1 is not None:
                    c1_row, r1_row = rank1
                    for m2 in range(8):
                        nc.tensor.matmul(
                            pss[m2][:],
                            c1_row[0:1, half * 1024 + m2 * P:
                                   half * 1024 + (m2 + 1) * P],
                            r1_row[:], start=False, stop=True,
                            skip_group_check=True)
                for m2 in range(8):
                    evict(half * 8 + m2, pss[m2])

        # ------------------------------------------------------------------
        # LN split: stats accumulate inside the producing GEMM's eviction;
        # ln_rows computes mu/rstd rows + broadcasts.  The normalize itself
        # is folded into the NEXT GEMM (diag(g)@W host-side + rank-1 corr).
        # ------------------------------------------------------------------
        def ln_stats(m, ctx, idx):
            if m == 0:
                ctx["mu"] = pg2h[0].tile([P, TOK], FP32, tag="mm", bufs=8,
                                         name=f"lnmu_ps{idx}")
                ctx["sq"] = pg2h[0].tile([P, TOK], FP32, tag="mm", bufs=8,
                                         name=f"lnsq_ps{idx}")
            src = ctx["src"][m]
            sq = sp.tile([P, TOK], BF16, tag="sq", bufs=3, name=f"lnsq{idx}_{m}")
            nc.vector.tensor_mul(sq[:], src[:], src[:])
            nc.tensor.matmul(ctx["mu"][0:1, :], onesb_col[:], src[:],
                             start=(m == 0), stop=(m == KC - 1),
                             skip_group_check=True)
            nc.tensor.matmul(ctx["sq"][0:1, :], onesb_col[:], sq[:],
                             start=(m == 0), stop=(m == KC - 1),
                             skip_group_check=True)

        def ln_rows(ctx, pp_unused, idx, want_mu_b=False):
            mu_ps, sq_ps = ctx["mu"], ctx["sq"]
            mu_row = sp.tile([1, TOK], FP32R, tag="row_r", bufs=2, name=f"lnmu{idx}")
            nc.scalar.activation(mu_row[:], mu_ps[0:1, :], AF.Copy, scale=1.0 / D)
            m2_row = sp.tile([1, TOK], FP32, tag="row", bufs=3, name=f"lnm2{idx}")
            nc.scalar.activation(m2_row[:], sq_ps[0:1, :], AF.Copy, scale=1.0 / D)
            var_row = sp.tile([1, TOK], FP32, tag="row", bufs=3, name=f"lnvar{idx}")
            musq = sp.tile([1, TOK], FP32, tag="row", bufs=3, name=f"lnmusq{idx}")
            nc.vector.tensor_mul(musq[:], mu_row[:], mu_row[:])
            nc.vector.tensor_sub(var_row[:], m2_row[:], musq[:])
            std_row = sp.tile([1, TOK], FP32, tag="row", bufs=3, name=f"lnstd{idx}")
            nc.scalar.activation(std_row[:], var_row[:], AF.Sqrt, bias=eps_t[:])
            rstd_row = sp.tile([1, TOK], FP32R, tag="row_r", bufs=2, name=f"lnrstd{idx}")
            nc.vector.reciprocal(rstd_row[:], std_row[:])
            # rank-1 correction row is plain mu: the eviction's rstd scale
            # multiplies (W'x - mu*colsum(W')) as a whole.
            return mu_row, rstd_row

        def ln_bcast(rows, holder, psum_pool, idx, want_mu_b):
            # emitted via post_k INSIDE the consuming GEMM so the in-order
            # tensor queue does not stall on the LN row math.
            mu_row, rstd_row = rows
            rs_bps = psum_pool.tile([P, TOK], FP32, tag="mm", bufs=8)
            nc.tensor.matmul(rs_bps[:], ones_row[:], rstd_row[:], start=True,
                             stop=True, skip_group_check=True)
            rs_b = sp.tile([P, TOK], FP32, tag="lnb", bufs=2, name=f"lnrsb{idx}")
            nc.vector.tensor_copy(rs_b[:], rs_bps[:])
            holder["rs_b"] = rs_b
            if want_mu_b:
                mu_bps = psum_pool.tile([P, TOK], FP32, tag="mm", bufs=8)
                nc.tensor.matmul(mu_bps[:], ones_row[:], mu_row[:], start=True,
                                 stop=True, skip_group_check=True)
                mu_b = sp.tile([P, TOK], FP32, tag="lnb", bufs=2, name=f"lnmub{idx}")
                nc.vector.tensor_copy(mu_b[:], mu_bps[:])
                holder["mu_b"] = mu_b

        # ==================================================================
        # phase 1: fused MoE (ONE bf16 GEMM) -> x3b (bf16) + x8 (fp8 packed)
        # ==================================================================
        pg2h = [None]
        with tc.tile_pool(name="pg", bufs=6, space="PSUM") as pg:
            pg2h[0] = pg

            def moe_evict(m, pss):
                nc.scalar.activation(x8[m // 2][:, (m % 2) * TOK:(m % 2 + 1) * TOK],
                                     pss[:], AF.Identity, scale=8.0,
                                     bias=moeb_t[:, m:m + 1])
                nc.scalar.activation(x3b[m][:], pss[:], AF.Identity,
                                     bias=moebf_t[:, m:m + 1])
            gemm_fm(io["moew"], D, D, xin, moe_evict, pg)

            # ==============================================================
            # phase 2: fp8 DoubleRow GEMMs, m-outer so evictions stream.
            # psum = sum (8 x3)(256 w) = 2048*(x3@w); evict scale 2^-7.
            # ==============================================================
            def qk_gemm(col0, evict):
                wts = []
                for kk in range(4):
                    wt = wp.tile([P, 2048], FP8, tag="w8", bufs=6)
                    (nc.sync if kk % 2 == 0 else nc.scalar).dma_start(
                        wt[:].rearrange("p (s c) -> p s c", s=2),
                        io["qkw8"][kk * 256:(kk + 1) * 256,
                                   col0:col0 + 1024].rearrange(
                                       "(s p) c -> p s c", p=P))
                    wts.append(wt[:].rearrange("p (s c) -> p s c", s=2))
                for m2 in range(8):
                    ps = pg.tile([P, TOK], FP32, tag="mm", bufs=8,
                                 name=f"psqk{col0}_{m2}")
                    for kk in range(4):
                        nc.tensor.matmul(
                            ps[:], wts[kk][:, :, m2 * P:(m2 + 1) * P],
                            x8[kk][:].rearrange("p (s c) -> p s c", s=2),
                            start=(kk == 0), stop=(kk == 3), perf_mode=DR)
                    evict(m2, ps)

            def k_evict(m, pss):
                # vector evict: (pss + 2048*kb) * 2^-7  = 16*(k + kb)
                k8 = sp.tile([P, TOK], FP8, tag="ev8", bufs=3, name=f"k8_{m}")
                nc.vector.tensor_scalar(k8[:], pss[:],
                                        qkb2048_t[:, 8 + m:9 + m], 2.0 ** -7,
                                        ALU.add, ALU.mult)
                h, s = m // 2, m % 2
                nc.gpsimd.dma_start(
                    kv_loc[h][s * 256:(s + 1) * 256, :].rearrange(
                        "(p a) c -> p (a c)", a=2),
                    k8[:])
            qk_gemm(1024, k_evict)

            # v token-major, (mt,n)-outer with resident weights; evict+store
            # per unit; AllGather h fires right after its last v store.
            vwts = []
            for kk in range(4):
                wt = wp.tile([P, 2048], FP8, tag="w8", bufs=6)
                (nc.sync if kk % 2 == 0 else nc.scalar).dma_start(
                    wt[:].rearrange("p (s c) -> p s c", s=2),
                    io["vw8"][kk * 256:(kk + 1) * 256, :].rearrange(
                        "(s p) c -> p s c", p=P))
                vwts.append(wt[:].rearrange("p (s c) -> p s c", s=2))
            for n in range(2):
                for mt in range(4):
                    ps = pg.tile([P, TOK], FP32, tag="mm", bufs=8,
                                 name=f"psv_{n}_{mt}")
                    for kk in range(4):
                        x83 = x8[kk][:].rearrange("p (s c) -> p s c", s=2)
                        nc.tensor.matmul(
                            ps[:], x83[:, :, mt * P:(mt + 1) * P],
                            vwts[kk][:, :, n * 512:(n + 1) * 512],
                            start=(kk == 0), stop=False, perf_mode=DR)
                    nc.tensor.matmul(ps[:], onesb_row[:],
                                     vb_row[0:1, n * 512:(n + 1) * 512],
                                     start=False, stop=True,
                                     skip_group_check=True)
                    v8 = sp.tile([P, TOK], FP8, tag="ev8", bufs=3,
                                 name=f"v8_{mt}_{n}")
                    nc.vector.tensor_scalar_mul(v8[:], ps[:], 2.0 ** -7)
                    for hh in range(2):
                        h = 2 * n + hh
                        nc.gpsimd.dma_start(
                            kv_loc[h][512 + mt * P:512 + (mt + 1) * P, :],
                            v8[:, hh * 256:(hh + 1) * 256])
                for hh in range(2):
                    h = 2 * n + hh
                    nc.gpsimd.collective_compute(
                        "AllGather", ALU.bypass,
                        replica_groups=[list(range(NCORES))],
                        ins=[kv_loc[h].opt()], outs=[kv_all[h].opt()])

            # q^T in fp8 (scaled x16), packed per head [P, 2*TOK]
            def q_evict(m, pss):
                h, s = m // 2, m % 2
                nc.vector.tensor_scalar(q8[h][:, s * TOK:(s + 1) * TOK], pss[:],
                                        qkb2048_t[:, m:m + 1], 2.0 ** -7,
                                        ALU.add, ALU.mult)
            qk_gemm(0, q_evict)

            # HAM heaters: keep the PE activity window busy while waiting
            # for the first AllGather (throwaway matmuls, never read).
            hps = pg.tile([P, TOK], FP32, tag="mm", bufs=8, name="heater")
            for i in range(60):
                nc.tensor.matmul(hps[:], x3b[0][:, 0:P], x3b[1][:],
                                 start=(i == 0), stop=(i == 59))

        # ==================================================================
        # phase 3: attention, one chunk per head.  st = 4096*S (DoubleRow);
        # es8 = 128*es (fp8); head output accumulates fully in PSUM, inline
        # epilogue per head; uniform part exact via host vrow.
        # ==================================================================
        with (
            tc.tile_pool(name="po", bufs=1, space="PSUM") as po,
            tc.tile_pool(name="ps_s", bufs=3, space="PSUM") as ps_s,
            tc.tile_pool(name="ps_t", bufs=1, space="PSUM") as ps_t,
        ):
            for h in range(H):
                # loads: 2 DMAs for all ktf (r-halves) + 1 for all vpf
                ktfall = sp.tile([P, NCORES * 2 * 512], FP8, tag="ktf", bufs=2,
                                 name=f"ktf{h}")
                kt4 = ktfall[:].rearrange("p (r s x) -> p r s x", r=NCORES, s=2)
                for rh in range(2):
                    for s in range(2):
                        nc.gpsimd.dma_start(
                            kt4[:, rh * 4:(rh + 1) * 4, s, :],
                            kv_all[h].rearrange("(r z) c -> r z c", z=1024)[
                                rh * 4:(rh + 1) * 4, s * 256:(s + 1) * 256,
                                :].rearrange("r (p a) c -> p r (a c)", a=2))
                vpfall = sp.tile([P, NCORES * 4 * 260], FP8, tag="vpf", bufs=1,
                                 name=f"vpf{h}")
                vp4 = vpfall[:].rearrange("p (r t x) -> p r t x", r=NCORES, t=4)
                for t in range(4):
                    nc.gpsimd.dma_start(
                        vp4[:, :, t, 0:256],
                        kv_all[h].rearrange("(r z) c -> r z c", z=1024)[
                            :, 512 + t * P:512 + (t + 1) * P, :].rearrange(
                                "r p c -> p r c"))
                nc.vector.memset(vp4[:, :, :, 256:260], 1.0)
                # scores + es for all (r, tsub)
                es8s = {}
                q83 = q8[h][:].rearrange("p (s c) -> p s c", s=2)
                idx = 0
                for r in range(NCORES):
                    for t in range(4):
                        st = ps_s.tile([P, TOK], FP32, tag="st")
                        nc.tensor.matmul(
                            st[:], kt4[:, r, :, t * P:(t + 1) * P], q83,
                            start=True, stop=True, perf_mode=DR)
                        tp = t // 2
                        if (r, tp) not in es8s:
                            es8s[(r, tp)] = sp.tile(
                                [P, 2 * TOK], FP8, tag="es8", bufs=16,
                                name=f"es8_{h}_{r}_{tp}")
                        dst = es8s[(r, tp)][:, (t % 2) * TOK:(t % 2 + 1) * TOK]
                        if idx % 4 != 3:
                            esf = sp.tile([P, TOK], FP32, tag="esf", bufs=2,
                                          name=f"esf{h}_{r}_{t}")
                            nc.scalar.activation(esf[:], st[:], AF.Exp,
                                                 scale=1.0 / 4096.0)
                            nc.vector.tensor_scalar(dst, esf[:], 128.0, 128.0,
                                                    ALU.mult, ALU.subtract)
                        else:
                            w = sp.tile([P, TOK], FP32, tag="esw", bufs=2,
                                        name=f"esw{h}_{r}_{t}")
                            nc.vector.tensor_scalar(w[:], st[:], 2.0 ** -18,
                                                    2.0 ** -5, ALU.mult, ALU.add)
                            nc.vector.tensor_mul(dst, w[:], st[:])
                        idx += 1
                # attn @ V accumulating over all 16 (r, tp) units per m
                o_ps = [po.tile([P, DH + 2], FP32, tag=f"o{m}", bufs=1,
                                name=f"ops{h}_{m}") for m in range(4)]
                for u, (r, tp) in enumerate([(r, tp) for r in range(NCORES)
                                             for tp in range(2)]):
                    es3 = es8s[(r, tp)][:].rearrange("p (s c) -> p s c", s=2)
                    for m in range(4):
                        nc.tensor.matmul(
                            o_ps[m][:], es3[:, :, m * P:(m + 1) * P],
                            vp4[:, r, 2 * tp:2 * tp + 2, 0:DH + 2],
                            start=(u == 0), stop=(u == 15),
                            perf_mode=DR, skip_group_check=True)
                # inline epilogue for this head
                bc_ps = ps_s.tile([P, DH + 2], FP32, tag="st", name=f"bc{h}")
                nc.tensor.matmul(bc_ps[:], ones_row[:],
                                 vrow_r[0:1, h * (DH + 2):(h + 1) * (DH + 2)],
                                 start=True, stop=True, skip_group_check=True)
                bc_sb = sp.tile([P, DH + 2], FP32, tag="bcs", bufs=2, name=f"bcs{h}")
                nc.vector.tensor_copy(bc_sb[:], bc_ps[:])
                for m in range(4):
                    of = sp.tile([P, DH + 2], FP32, tag="of", bufs=2, name=f"of{h}_{m}")
                    nc.vector.tensor_add(of[:], bc_sb[:], o_ps[m][:])
                    recip = sp.tile([P, 1], FP32, tag="rc", bufs=2, name=f"rc{h}_{m}")
                    nc.vector.reciprocal(recip[:], of[:, DH:DH + 1])
                    osc = sp.tile([P, DH], FP32R, tag="osc", bufs=2, name=f"osc{h}_{m}")
                    nc.vector.tensor_scalar_mul(osc[:], of[:, 0:DH], recip[:])
                    for d2 in range(2):
                        tp_ = ps_t.tile([P, P], FP32R, tag="tp")
                        nc.tensor.transpose(tp_[:], osc[:, d2 * P:(d2 + 1) * P], eye[:])
                        nc.vector.tensor_copy(
                            oT[2 * h + d2][:, m * P:(m + 1) * P], tp_[:])

        # ==================================================================
        # phase 4: o-proj + folded-LN FFN + folded-LN trailing stack
        # ==================================================================
        with tc.tile_pool(name="pg2", bufs=6, space="PSUM") as pg2:
            pg2h[0] = pg2
            ln1ctx, ln2ctx = {"src": xRb}, {"src": xR2b}
            q3 = [nc.sync, nc.scalar, nc.gpsimd]

            def oproj_evict(m, pss):
                t = sp.tile([P, TOK], BF16, tag="ev16", bufs=3, name=f"op{m}")
                nc.scalar.activation(t[:], pss[:], AF.Identity,
                                     bias=ob_t[:, m:m + 1])
                nc.vector.tensor_add(xRb[m][:], x3b[m][:], t[:])
                ln_stats(m, ln1ctx, 0)
            gemm_fm(io["ow"], D, D, oT, oproj_evict, pg2, m_outer=True)
            rows1 = ln_rows(ln1ctx, pg2, 0, want_mu_b=True)
            hold1 = {}

            # f1 on the RAW residual (LN folded into f1w' + rank-1 + scale)
            def f1_evict(m, pss):
                tv = sp.tile([P, TOK], FP32, tag="evf", bufs=2, name=f"f1v{m}")
                nc.vector.tensor_mul(tv[:], hold1["rs_b"][:], pss[:])
                nc.scalar.activation(hT[m][:], tv[:], AF.Relu,
                                     bias=f1b_t[:, m:m + 1])
            ln_bcast(rows1, hold1, pg2, 0, True)
            gemm_fm(io["f1w"], D, DFF, xRb, f1_evict, pg2, engines=q3,
                    rank1=(c1f1_r, rows1[0]))
            # lazy-materialize yA = LN1(x) for the f2 residual add
            for k in range(KC):
                t1 = sp.tile([P, TOK], FP32, tag="ev", bufs=2, name=f"lnt1_{k}")
                nc.vector.tensor_sub(t1[:], xRb[k][:], hold1["mu_b"][:])
                t2 = sp.tile([P, TOK], FP32, tag="ev", bufs=2, name=f"lnt2_{k}")
                nc.vector.tensor_mul(t2[:], t1[:], hold1["rs_b"][:])
                nc.scalar.activation(yA[k][:], t2[:], AF.Identity,
                                     scale=ln1g_t[:, k:k + 1],
                                     bias=ln1b_t[:, k:k + 1])

            def f2_evict(m, pss):
                t = sp.tile([P, TOK], BF16, tag="ev16", bufs=3, name=f"f2e{m}")
                nc.scalar.activation(t[:], pss[:], AF.Identity,
                                     bias=f2b_t[:, m:m + 1])
                nc.vector.tensor_add(xR2b[m][:], yA[m][:], t[:])
                ln_stats(m, ln2ctx, 1)
            gemm_fm(io["f2w"], DFF, D, hT, f2_evict, pg2, engines=q3)
            rows2 = ln_rows(ln2ctx, pg2, 1, want_mu_b=False)
            hold2 = {}

            # fc on the raw second residual (LN2 fully folded)
            def fc_evict(m, pss):
                tv = sp.tile([P, TOK], FP32, tag="evf", bufs=2, name=f"fcv{m}")
                nc.vector.tensor_mul(tv[:], hold2["rs_b"][:], pss[:])
                nc.scalar.activation(yA[m][:], tv[:], AF.Identity,
                                     bias=fcb_t[:, m:m + 1])
            ln_bcast(rows2, hold2, pg2, 1, False)
            gemm_fm(io["fcw"], D, D, xR2b, fc_evict, pg2, engines=q3,
                    rank1=(c1fc_r, rows2[0]))

            def mk_evict(out_tiles, bias_t, relu=False):
                def ev(m, pss):
                    nc.scalar.activation(out_tiles[m][:], pss[:],
                                         AF.Relu if relu else AF.Identity,
                                         bias=bias_t[:, m:m + 1])
                return ev
            gemm_fm(io["k1w"], D, D, yA, mk_evict(yB, k1b_t, relu=True), pg2,
                    engines=q3)

            def out_evict(m, pss):
                fin = sp.tile([P, TOK], FP32, tag="ev", bufs=2, name=f"fin{m}")
                nc.scalar.activation(fin[:], pss[:], AF.Identity,
                                     bias=kob_t[:, m:m + 1])
                q3[m % 3].dma_start(io["outT"][m * P:(m + 1) * P, :], fin[:])
            gemm_fm(io["kow"], D, D, yB, out_evict, pg2, engines=q3,
                    m_outer=True)


def _build():
    nc = bacc.Bacc("TRN2", debug=False, num_devices=NCORES)

    def din(name, shape, dt=BF16):
        return nc.dram_tensor(name, shape, dt, kind="ExternalInput").ap()

    io = {
        "xT": din("xT", [D, TOK]),
        "moew": din("moew", [D, D]),
        "qkw8": din("qkw8", [D, 2 * D], FP8),
        "vw8": din("vw8", [D, D], FP8),
        "vb2048": din("vb2048", [D]),
        "ow": din("ow", [D, D]),
        "f1w": din("f1w", [D, DFF]),
        "f2w": din("f2w", [DFF, D]),
        "fcw": din("fcw", [D, D]),
        "k1w": din("k1w", [D, D]),
        "kow": din("kow", [D, D]),
        "c_onesb": din("c_onesb", [1024], BF16),
        "c_ones": din("c_ones", [256], FP32),
        "c_eye": din("c_eye", [128, 128], FP32),
        "vrow": din("vrow", [H * (DH + 2)], FP32),
        "c1f1": din("c1f1", [DFF], FP32),
        "c1fc": din("c1fc", [D], FP32),
    }
    for name, shape in [("qkb16", [2 * D]), ("qkb2048", [2 * D]),
                        ("ob", [D]), ("f1b", [DFF]),
                        ("f2b", [D]), ("ln1g", [D]), ("ln1b", [D]),
                        ("fcb", [D]), ("k1b", [D]), ("kob", [D]),
                        ("moeb", [D]), ("moeb8", [D])]:
        io[name] = din(name, shape, FP32)
    io["outT"] = nc.dram_tensor("outT", [D, TOK], FP32, kind="ExternalOutput").ap()

    with nc.allow_low_precision("bf16/fp8 matmul pipeline"):
        with tile.TileContext(nc) as tc:
            _body(nc, tc, io)
    nc.compile()
    return nc


# ----------------------------------------------------------------------------
# host side
# ----------------------------------------------------------------------------

def kernel(x, gw, gb, ew, eb, qkvw, qkvb, ow, ob, ln1g, ln1b, ln2g, ln2b,
           f1w, f1b, f2w, f2b, ffw, ffb, cfw, cfb, k1w, k1b, k2w, k2b,
           outw, outb):
    f64 = np.float64
    bf16 = ml_dtypes.bfloat16
    fp8 = ml_dtypes.float8_e4m3
    x = np.asarray(x, np.float32)
    gw, gb = np.asarray(gw, np.float32), np.asarray(gb, np.float32)
    ew, eb = np.asarray(ew, np.float32), np.asarray(eb, np.float32)
    qkvw, qkvb = np.asarray(qkvw, np.float32), np.asarray(qkvb, np.float32)

    # degenerate routing (token 0) + MoE layer fusion, all in f64
    x0 = x[0].astype(f64)
    Ws, bs = [], []
    for l in range(L):
        s = x0 @ gw[l].astype(f64) + gb[l].astype(f64)
        sel = np.argsort(-s, kind="stable")[:2]
        W = (ew[l][sel[0]].astype(f64) + ew[l][sel[1]].astype(f64)) * 0.5
        b = (eb[l][sel[0]].astype(f64) + eb[l][sel[1]].astype(f64)) * 0.5
        x0 = x0 @ W + b
        Ws.append(W)
        bs.append(b)
    Wf = Ws[0] @ Ws[1] @ Ws[2]
    bf_ = bs[0] @ Ws[1] @ Ws[2] + bs[1] @ Ws[2] + bs[2]

    # exact column sums of v for the attention uniform part
    vw_ = qkvw[:, 2 * D:].astype(f64)
    vb_ = qkvb[2 * D:].astype(f64)
    colx3 = x.astype(f64).sum(0) @ Wf + N * bf_
    vsum = colx3 @ vw_ + N * vb_                       # [D]
    vrow = np.zeros((H, DH + 2), np.float32)
    for h in range(H):
        vrow[h, :DH] = (2048.0 * vsum[h * DH:(h + 1) * DH]).astype(np.float32)
        vrow[h, DH] = 128.0 * N
    # LN-folded weights: f1w' = diag(ln1g) @ f1w, etc.
    ln1g64 = np.asarray(ln1g, f64)
    ln2g64 = np.asarray(ln2g, f64)
    f1wp = ln1g64[:, None] * np.asarray(f1w, f64)
    f1bp = np.asarray(ln1b, f64) @ np.asarray(f1w, f64) + np.asarray(f1b, f64)
    c1f1 = -f1wp.sum(0)
    Wfc = np.asarray(ffw, f64) @ np.asarray(cfw, f64)
    bfc = np.asarray(ffb, f64) @ np.asarray(cfw, f64) + np.asarray(cfb, f64)
    fcwp = ln2g64[:, None] * Wfc
    fcbp = np.asarray(ln2b, f64) @ Wfc + bfc
    c1fc = -fcwp.sum(0)
    Wko = np.asarray(k2w, f64) @ np.asarray(outw, f64)
    bko = np.asarray(k2b, f64) @ np.asarray(outw, f64) + np.asarray(outb, f64)

    if "nc" not in _CACHE:
        _CACHE["nc"] = _build()
    nc = _CACHE["nc"]

    shared = {
        "moew": Wf.astype(bf16), "moeb": bf_.astype(np.float32),
        "moeb8": (bf_ * 8.0).astype(np.float32),
        "qkw8": np.clip(np.ascontiguousarray(qkvw[:, :2 * D]) * 256.0,
                        -240, 240).astype(fp8),
        "qkb16": (qkvb[:2 * D] * 16.0).astype(np.float32),
        "qkb2048": (qkvb[:2 * D] * 2048.0).astype(np.float32),
        "vw8": np.clip(np.ascontiguousarray(vw_) * 256.0, -240, 240).astype(fp8),
        "vb2048": (vb_ * 2048.0).astype(bf16),
        "ow": (np.asarray(ow, np.float32) / 16.0).astype(bf16),
        "ob": np.asarray(ob, np.float32),
        "f1w": f1wp.astype(bf16),
        "f1b": f1bp.astype(np.float32),
        "f2w": np.asarray(f2w, np.float32).astype(bf16),
        "f2b": np.asarray(f2b, np.float32),
        "ln1g": np.asarray(ln1g, np.float32), "ln1b": np.asarray(ln1b, np.float32),
        "fcw": fcwp.astype(bf16), "fcb": fcbp.astype(np.float32),
        "k1w": np.asarray(k1w, np.float32).astype(bf16),
        "k1b": np.asarray(k1b, np.float32),
        "kow": Wko.astype(bf16), "kob": bko.astype(np.float32),
        "c_onesb": np.ones(1024, bf16),
        "c_ones": np.ones(256, np.float32),
        "c_eye": np.eye(128, dtype=np.float32),
        "vrow": vrow.reshape(-1),
        "c1f1": c1f1.astype(np.float32),
        "c1fc": c1fc.astype(np.float32),
    }

    in_maps = []
    for c in range(NCORES):
        m = dict(shared)
        m["xT"] = np.ascontiguousarray(x[c * TOK:(c + 1) * TOK].T).astype(bf16)
        in_maps.append(m)

    _CACHE["in_maps"] = in_maps
    res = bass_utils.run_bass_kernel_spmd(nc, in_maps, core_ids=list(range(NCORES)))
    _CACHE["last_result"] = res

    out = np.empty((N, D), np.float32)
    for c in range(NCORES):
        out[c * TOK:(c + 1) * TOK, :] = res.results[c]["outT"].T
    return out


# revision 51
# speedup vs baseline: 1.0812x; 1.0812x over previous
"""Trainium2 Bass kernel for nn_LiquidModel (moe_routing) — v5.

Host does all algebraic folding; device does 12 GEMM-units + attention:
 - Degenerate MoE routing (experts picked by token 0, averaged) -> the three
   MoE layers fold into ONE dense GEMM (W1@W2@W3, f64 on host).  Same for
   ffw@cfw and k2w@outw in the trailing stack.
 - Dense GEMMs run bf16 (fp32 PSUM accumulate); residual stream is bf16.
 - q/k/v GEMMs run fp8 DoubleRow (256-deep contraction per instruction);
   their outputs are fp8-quantized for the exchange anyway.
 - K/V exchange: 4 AllGathers, one per HEAD (k head-slice [256,512] + v
   head-slice [512,256], fp8, 256KB payload).  Attention chunk h consumes
   AllGather h; the head's output accumulates fully in PSUM (no SBUF
   folds) and its epilogue runs inline.
 - Softmax uniform part is EXACT: sum_t v comes from the host column-sum
   identity, so fp8 noise only touches the small es*v correction term.
 - LN1/LN2 are folded into the following GEMM (diag(g)@W on host + rank-1
   mu*rstd correction matmul + per-token rstd scale at eviction), so the
   tensor engine never waits for LN row math.
"""
import ml_dtypes
import numpy as np

import concourse.bacc as bacc
import concourse.bass as bass
import concourse.mybir as mybir
import concourse.tile as tile
from concourse import bass_utils

FP32 = mybir.dt.float32
FP32R = mybir.dt.float32r
BF16 = mybir.dt.bfloat16
FP8 = mybir.dt.float8e4
AF = mybir.ActivationFunctionType
ALU = mybir.AluOpType
DR = mybir.MatmulPerfMode.DoubleRow

NCORES = 8
N, D, DFF, H, L = 4096, 1024, 2048, 4, 3
TOK = N // NCORES          # 512 tokens per core
DH = D // H                # 256
EPS = 1e-5
KC = D // 128              # 8 feature chunks of 128
P = 128

_CACHE = {}


def _body(nc, tc, io):
    # ---- persistent SBUF activation tensors (feature-major [128, TOK]) ----
    xin = [nc.alloc_sbuf_tensor(f"xin{i}", [P, TOK], BF16).ap() for i in range(KC)]
    x3b = [nc.alloc_sbuf_tensor(f"x3b{i}", [P, TOK], BF16).ap() for i in range(KC)]
    x8 = [nc.alloc_sbuf_tensor(f"x8_{i}", [P, 2 * TOK], FP8).ap() for i in range(KC // 2)]
    xRb = [nc.alloc_sbuf_tensor(f"xRb{i}", [P, TOK], BF16).ap() for i in range(KC)]
    yA = [nc.alloc_sbuf_tensor(f"yA{i}", [P, TOK], BF16).ap() for i in range(KC)]
    yB = [nc.alloc_sbuf_tensor(f"yB{i}", [P, TOK], BF16).ap() for i in range(KC)]
    hT = [nc.alloc_sbuf_tensor(f"hT{i}", [P, TOK], BF16).ap() for i in range(2 * KC)]
    q8 = [nc.alloc_sbuf_tensor(f"q8_{h}", [P, 2 * TOK], FP8).ap() for h in range(H)]
    oT = xin      # attention output reuses the input slots (dead after MoE)
    xR2b = xin    # and the second residual reuses them again after o-proj

    with (
        tc.tile_pool(name="const", bufs=1) as cp,
        tc.tile_pool(name="wp", bufs=10) as wp,
        tc.tile_pool(name="sp", bufs=4) as sp,
        tc.tile_pool(name="dram", bufs=1, space="DRAM") as dp,
    ):
        # ---- input x loads first (gpsimd queue, feeds the MoE GEMM asap) ----
        for i in range(KC):
            nc.gpsimd.dma_start(xin[i][:], io["xT"][i * P:(i + 1) * P, :])

        # ---- constants (gpsimd queue) ----
        onesb_col = cp.tile([P, 1], BF16, tag="onesb_col")
        nc.gpsimd.dma_start(onesb_col[:], io["c_onesb"][0:128].rearrange("(p o) -> p o", o=1))
        onesb_row = cp.tile([1, P], BF16, tag="onesb_row")
        nc.gpsimd.dma_start(onesb_row[:], io["c_onesb"][0:128].rearrange("(o p) -> o p", o=1))
        ones_row = cp.tile([1, P], FP32R, tag="ones_row")
        nc.gpsimd.dma_start(ones_row[:], io["c_ones"][0:128].rearrange("(o p) -> o p", o=1))
        eye = cp.tile([P, P], FP32R, tag="eye")
        nc.gpsimd.dma_start(eye[:], io["c_eye"][:, :])
        eps_t = cp.tile([1, 1], FP32, tag="eps")
        nc.vector.memset(eps_t[:], EPS)
        vb_row = cp.tile([1, D], BF16, tag="vb_row")
        nc.gpsimd.dma_start(vb_row[:], io["vb2048"][:].rearrange("(o d) -> o d", o=1))
        vrow_r = cp.tile([1, H * (DH + 2)], FP32R, tag="vrow")
        nc.gpsimd.dma_start(vrow_r[:], io["vrow"][:].rearrange("(o d) -> o d", o=1))
        c1f1_r = cp.tile([1, DFF], FP32R, tag="c1f1")
        nc.gpsimd.dma_start(c1f1_r[:], io["c1f1"][:].rearrange("(o d) -> o d", o=1))
        c1fc_r = cp.tile([1, D], FP32R, tag="c1fc")
        nc.gpsimd.dma_start(c1fc_r[:], io["c1fc"][:].rearrange("(o d) -> o d", o=1))

        def vec_tile(name, length):
            cols = length // P
            t = cp.tile([P, cols], FP32, tag=f"vec_{name}")
            nc.gpsimd.dma_start(t[:], io[name][:].rearrange("(c p) -> p c", p=P))
            return t

        moeb_t = vec_tile("moeb8", D)       # 8*moeb (for the fp8 x8 evict)
        moebf_t = vec_tile("moeb", D)       # moeb   (for the bf16 x3 evict)
        qkb16_t = vec_tile("qkb16", 2 * D)
        ob_t = vec_tile("ob", D)
        f1b_t = vec_tile("f1b", DFF)        # ln1b@f1w + f1b (host-folded)
        f2b_t = vec_tile("f2b", D)
        ln1g_t = vec_tile("ln1g", D)
        ln1b_t = vec_tile("ln1b", D)
        fcb_t = vec_tile("fcb", D)          # ln2b@fcw + bfc (host-folded)
        k1b_t = vec_tile("k1b", D)
        kob_t = vec_tile("kob", D)

        # ---- DRAM buffers: one exchange chunk per HEAD ----
        # kv_loc[h] = [1024, 256] fp8: rows 0..511 = k head-slice (row =
        # s*256 + p*2 + a: feature s*128+p, token-half a); rows 512..1023 =
        # v head-slice, one row per token.
        kv_loc = [dp.tile([1024, 256], FP8, tag=f"kv_loc{h}", name=f"kv_loc{h}")
                  for h in range(H)]
        kv_all = [dp.tile([NCORES * 1024, 256], FP8, tag=f"kv_all{h}",
                          name=f"kv_all{h}", addr_space="Shared")
                  for h in range(H)]

        # ------------------------------------------------------------------
        # dense feature-major GEMM:  psum[M-chunk, TOK] = W[K, M]^T-contr x^T
        # evict(m, pss) per output chunk m.  rank1=(c1_row, r1_getter) adds
        # the LN-fold rank-1 correction before closing the accumulation.
        # ------------------------------------------------------------------
        def gemm_fm(w_ap, K, M, x_tiles, evict, psum_pool, engines=None,
                    rank1=None, m_outer=False):
            engines = engines or [nc.sync, nc.scalar]
            kc = K // P
            for half in range(M // 1024):
                if m_outer:
                    # resident weights; each m-chunk closes early so the
                    # eviction chain pipelines under the remaining matmuls
                    wts = []
                    for k in range(kc):
                        wt = wp.tile([P, 1024], BF16, tag="w", bufs=12)
                        engines[k % len(engines)].dma_start(
                            wt[:], w_ap[k * P:(k + 1) * P,
                                        half * 1024:(half + 1) * 1024])
                        wts.append(wt)
                    for m2 in range(8):
                        ps = psum_pool.tile([P, TOK], FP32, tag="mm", bufs=8,
                                            name=f"psmo{half}_{m2}")
                        for k in range(kc):
                            nc.tensor.matmul(
                                ps[:], wts[k][:, m2 * P:(m2 + 1) * P],
                                x_tiles[k][:], start=(k == 0),
                                stop=(k == kc - 1 and rank1 is None))
                        if rank1 is not None:
                            c1_row, r1_row = rank1
                            nc.tensor.matmul(
                                ps[:],
                                c1_row[0:1, half * 1024 + m2 * P:
                                       half * 1024 + (m2 + 1) * P],
                                r1_row[:], start=False, stop=True,
                                skip_group_check=True)
                        evict(half * 8 + m2, ps)
                    continue
                pss = [psum_pool.tile([P, TOK], FP32, tag="mm", bufs=8,
                                      name=f"ps{half}_{i}") for i in range(8)]
                for k in range(kc):
                    wt = wp.tile([P, 1024], BF16, tag="w", bufs=12)
                    engines[k % len(engines)].dma_start(
                        wt[:],
                        w_ap[k * P:(k + 1) * P,
                             half * 1024:(half + 1) * 1024])
                    for m2 in range(8):
                        nc.tensor.matmul(
                            pss[m2][:], wt[:, m2 * P:(m2 + 1) * P],
                            x_tiles[k][:],
                            start=(k == 0),
                            stop=(k == kc - 1 and rank1 is None))
                if rank1 is not None:
                    c1_row, r1_row = rank1
                    for m2 in range(8):
                        nc.tensor.matmul(
                            pss[m2][:],
                            c1_row[0:1, half * 1024 + m2 * P:
                                   half * 1024 + (m2 + 1) * P],
                            r1_row[:], start=False, stop=True,
                            skip_group_check=True)
                for m2 in range(8):
                    evict(half * 8 + m2, pss[m2])

        # ------------------------------------------------------------------
        # LN split: stats accumulate inside the producing GEMM's eviction;
        # ln_rows computes mu/rstd rows + broadcasts.  The normalize itself
        # is folded into the NEXT GEMM (diag(g)@W host-side + rank-1 corr).
        # ------------------------------------------------------------------
        def ln_stats(m, ctx, idx):
            if m == 0:
                ctx["mu"] = pg2h[0].tile([P, TOK], FP32, tag="mm", bufs=8,
                                         name=f"lnmu_ps{idx}")
                ctx["sq"] = pg2h[0].tile([P, TOK], FP32, tag="mm", bufs=8,
                                         name=f"lnsq_ps{idx}")
            src = ctx["src"][m]
            sq = sp.tile([P, TOK], BF16, tag="sq", bufs=3, name=f"lnsq{idx}_{m}")
            nc.vector.tensor_mul(sq[:], src[:], src[:])
            nc.tensor.matmul(ctx["mu"][0:1, :], onesb_col[:], src[:],
                             start=(m == 0), stop=(m == KC - 1),
                             skip_group_check=True)
            nc.tensor.matmul(ctx["sq"][0:1, :], onesb_col[:], sq[:],
                             start=(m == 0), stop=(m == KC - 1),
                             skip_group_check=True)

        def ln_rows(ctx, pp_unused, idx, want_mu_b=False):
            mu_ps, sq_ps = ctx["mu"], ctx["sq"]
            mu_row = sp.tile([1, TOK], FP32R, tag="row_r", bufs=2, name=f"lnmu{idx}")
            nc.scalar.activation(mu_row[:], mu_ps[0:1, :], AF.Copy, scale=1.0 / D)
            m2_row = sp.tile([1, TOK], FP32, tag="row", bufs=3, name=f"lnm2{idx}")
            nc.scalar.activation(m2_row[:], sq_ps[0:1, :], AF.Copy, scale=1.0 / D)
            var_row = sp.tile([1, TOK], FP32, tag="row", bufs=3, name=f"lnvar{idx}")
            musq = sp.tile([1, TOK], FP32, tag="row", bufs=3, name=f"lnmusq{idx}")
            nc.vector.tensor_mul(musq[:], mu_row[:], mu_row[:])
            nc.vector.tensor_sub(var_row[:], m2_row[:], musq[:])
            std_row = sp.tile([1, TOK], FP32, tag="row", bufs=3, name=f"lnstd{idx}")
            nc.scalar.activation(std_row[:], var_row[:], AF.Sqrt, bias=eps_t[:])
            rstd_row = sp.tile([1, TOK], FP32R, tag="row_r", bufs=2, name=f"lnrstd{idx}")
            nc.vector.reciprocal(rstd_row[:], std_row[:])
            # rank-1 correction row is plain mu: the eviction's rstd scale
            # multiplies (W'x - mu*colsum(W')) as a whole.
            return mu_row, rstd_row

        def ln_bcast(rows, holder, psum_pool, idx, want_mu_b):
            # emitted via post_k INSIDE the consuming GEMM so the in-order
            # tensor queue does not stall on the LN row math.
            mu_row, rstd_row = rows
            rs_bps = psum_pool.tile([P, TOK], FP32, tag="mm", bufs=8)
            nc.tensor.matmul(rs_bps[:], ones_row[:], rstd_row[:], start=True,
                             stop=True, skip_group_check=True)
            rs_b = sp.tile([P, TOK], FP32, tag="lnb", bufs=2, name=f"lnrsb{idx}")
            nc.vector.tensor_copy(rs_b[:], rs_bps[:])
            holder["rs_b"] = rs_b
            if want_mu_b:
                mu_bps = psum_pool.tile([P, TOK], FP32, tag="mm", bufs=8)
                nc.tensor.matmul(mu_bps[:], ones_row[:], mu_row[:], start=True,
                                 stop=True, skip_group_check=True)
                mu_b = sp.tile([P, TOK], FP32, tag="lnb", bufs=2, name=f"lnmub{idx}")
                nc.vector.tensor_copy(mu_b[:], mu_bps[:])
                holder["mu_b"] = mu_b

        # ==================================================================
        # phase 1: fused MoE (ONE bf16 GEMM) -> x3b (bf16) + x8 (fp8 packed)
        # ==================================================================
        pg2h = [None]
        with tc.tile_pool(name="pg", bufs=6, space="PSUM") as pg:
            pg2h[0] = pg

            def moe_evict(m, pss):
                nc.scalar.activation(x8[m // 2][:, (m % 2) * TOK:(m % 2 + 1) * TOK],
                                     pss[:], AF.Identity, scale=8.0,
                                     bias=moeb_t[:, m:m + 1])
                nc.scalar.activation(x3b[m][:], pss[:], AF.Identity,
                                     bias=moebf_t[:, m:m + 1])
            gemm_fm(io["moew"], D, D, xin, moe_evict, pg)

            # ==============================================================
            # phase 2: fp8 DoubleRow GEMMs, m-outer so evictions stream.
            # psum = sum (8 x3)(256 w) = 2048*(x3@w); evict scale 2^-7.
            # ==============================================================
            def qk_gemm(col0, evict):
                wts = []
                for kk in range(4):
                    wt = wp.tile([P, 2048], FP8, tag="w8", bufs=6)
                    (nc.sync if kk % 2 == 0 else nc.scalar).dma_start(
                        wt[:].rearrange("p (s c) -> p s c", s=2),
                        io["qkw8"][kk * 256:(kk + 1) * 256,
                                   col0:col0 + 1024].rearrange(
                                       "(s p) c -> p s c", p=P))
                    wts.append(wt[:].rearrange("p (s c) -> p s c", s=2))
                for m2 in range(8):
                    ps = pg.tile([P, TOK], FP32, tag="mm", bufs=8,
                                 name=f"psqk{col0}_{m2}")
                    for kk in range(4):
                        nc.tensor.matmul(
                            ps[:], wts[kk][:, :, m2 * P:(m2 + 1) * P],
                            x8[kk][:].rearrange("p (s c) -> p s c", s=2),
                            start=(kk == 0), stop=(kk == 3), perf_mode=DR)
                    evict(m2, ps)

            # v FIRST (token-major, (n,mt)-outer): its stores complete all
            # of kv_loc's v-sections before the k GEMM runs.
            vwts = []
            for kk in range(4):
                wt = wp.tile([P, 2048], FP8, tag="w8", bufs=6)
                (nc.sync if kk % 2 == 0 else nc.scalar).dma_start(
                    wt[:].rearrange("p (s c) -> p s c", s=2),
                    io["vw8"][kk * 256:(kk + 1) * 256, :].rearrange(
                        "(s p) c -> p s c", p=P))
                vwts.append(wt[:].rearrange("p (s c) -> p s c", s=2))
            for n in range(2):
                for mt in range(4):
                    ps = pg.tile([P, TOK], FP32, tag="mm", bufs=8,
                                 name=f"psv_{n}_{mt}")
                    for kk in range(4):
                        x83 = x8[kk][:].rearrange("p (s c) -> p s c", s=2)
                        nc.tensor.matmul(
                            ps[:], x83[:, :, mt * P:(mt + 1) * P],
                            vwts[kk][:, :, n * 512:(n + 1) * 512],
                            start=(kk == 0), stop=False, perf_mode=DR)
                    nc.tensor.matmul(ps[:], onesb_row[:],
                                     vb_row[0:1, n * 512:(n + 1) * 512],
                                     start=False, stop=True,
                                     skip_group_check=True)
                    v8 = sp.tile([P, TOK], FP8, tag="ev8", bufs=3,
                                 name=f"v8_{mt}_{n}")
                    nc.scalar.activation(v8[:], ps[:], AF.Identity,
                                         scale=2.0 ** -7)
                    for hh in range(2):
                        h = 2 * n + hh
                        nc.gpsimd.dma_start(
                            kv_loc[h][512 + mt * P:512 + (mt + 1) * P, :],
                            v8[:, hh * 256:(hh + 1) * 256])

            # k (m-outer DR GEMM): every odd m-chunk eviction completes one
            # head's kv_loc, so its AllGather fires immediately.
            def k_evict(m, pss):
                k8 = sp.tile([P, TOK], FP8, tag="ev8", bufs=3, name=f"k8_{m}")
                nc.scalar.activation(k8[:], pss[:], AF.Identity, scale=2.0 ** -7,
                                     bias=qkb16_t[:, 8 + m:9 + m])
                h, s = m // 2, m % 2
                nc.gpsimd.dma_start(
                    kv_loc[h][s * 256:(s + 1) * 256, :].rearrange(
                        "(p a) c -> p (a c)", a=2),
                    k8[:])
                if s == 1:
                    nc.gpsimd.collective_compute(
                        "AllGather", ALU.bypass,
                        replica_groups=[list(range(NCORES))],
                        ins=[kv_loc[h].opt()], outs=[kv_all[h].opt()])
            qk_gemm(1024, k_evict)

            # q^T in fp8 (scaled x16), packed per head [P, 2*TOK]
            def q_evict(m, pss):
                h, s = m // 2, m % 2
                nc.scalar.activation(q8[h][:, s * TOK:(s + 1) * TOK], pss[:],
                                     AF.Identity, scale=2.0 ** -7,
                                     bias=qkb16_t[:, m:m + 1])
            qk_gemm(0, q_evict)

        # ==================================================================
        # phase 3: attention, one chunk per head.  st = 4096*S (DoubleRow);
        # es8 = 128*es (fp8); head output accumulates fully in PSUM, inline
        # epilogue per head; uniform part exact via host vrow.
        # ==================================================================
        with (
            tc.tile_pool(name="po", bufs=1, space="PSUM") as po,
            tc.tile_pool(name="ps_s", bufs=3, space="PSUM") as ps_s,
            tc.tile_pool(name="ps_t", bufs=1, space="PSUM") as ps_t,
        ):
            for h in range(H):
                # loads: 2 DMAs for all ktf (r-halves) + 1 for all vpf
                ktfall = sp.tile([P, NCORES * 2 * 512], FP8, tag="ktf", bufs=2,
                                 name=f"ktf{h}")
                kt4 = ktfall[:].rearrange("p (r s x) -> p r s x", r=NCORES, s=2)
                for rh in range(2):
                    for s in range(2):
                        nc.gpsimd.dma_start(
                            kt4[:, rh * 4:(rh + 1) * 4, s, :],
                            kv_all[h].rearrange("(r z) c -> r z c", z=1024)[
                                rh * 4:(rh + 1) * 4, s * 256:(s + 1) * 256,
                                :].rearrange("r (p a) c -> p r (a c)", a=2))
                vpfall = sp.tile([P, NCORES * 4 * 260], FP8, tag="vpf", bufs=1,
                                 name=f"vpf{h}")
                vp4 = vpfall[:].rearrange("p (r t x) -> p r t x", r=NCORES, t=4)
                for t in range(4):
                    nc.gpsimd.dma_start(
                        vp4[:, :, t, 0:256],
                        kv_all[h].rearrange("(r z) c -> r z c", z=1024)[
                            :, 512 + t * P:512 + (t + 1) * P, :].rearrange(
                                "r p c -> p r c"))
                nc.vector.memset(vp4[:, :, :, 256:260], 1.0)
                # scores + es for all (r, tsub)
                es8s = {}
                q83 = q8[h][:].rearrange("p (s c) -> p s c", s=2)
                idx = 0
                for r in range(NCORES):
                    for t in range(4):
                        st = ps_s.tile([P, TOK], FP32, tag="st")
                        nc.tensor.matmul(
                            st[:], kt4[:, r, :, t * P:(t + 1) * P], q83,
                            start=True, stop=True, perf_mode=DR)
                        tp = t // 2
                        if (r, tp) not in es8s:
                            es8s[(r, tp)] = sp.tile(
                                [P, 2 * TOK], FP8, tag="es8", bufs=16,
                                name=f"es8_{h}_{r}_{tp}")
                        dst = es8s[(r, tp)][:, (t % 2) * TOK:(t % 2 + 1) * TOK]
                        if idx % 4 != 3:
                            esf = sp.tile([P, TOK], FP32, tag="esf", bufs=2,
                                          name=f"esf{h}_{r}_{t}")
                            nc.scalar.activation(esf[:], st[:], AF.Exp,
                                                 scale=1.0 / 4096.0)
                            nc.vector.tensor_scalar(dst, esf[:], 128.0, 128.0,
                                                    ALU.mult, ALU.subtract)
                        else:
                            w = sp.tile([P, TOK], FP32, tag="esw", bufs=2,
                                        name=f"esw{h}_{r}_{t}")
                            nc.vector.tensor_scalar(w[:], st[:], 2.0 ** -18,
                                                    2.0 ** -5, ALU.mult, ALU.add)
                            nc.vector.tensor_mul(dst, w[:], st[:])
                        idx += 1
                # attn @ V accumulating over all 16 (r, tp) units per m
                o_ps = [po.tile([P, DH + 2], FP32, tag=f"o{m}", bufs=1,
                                name=f"ops{h}_{m}") for m in range(4)]
                for u, (r, tp) in enumerate([(r, tp) for r in range(NCORES)
                                             for tp in range(2)]):
                    es3 = es8s[(r, tp)][:].rearrange("p (s c) -> p s c", s=2)
                    for m in range(4):
                        nc.tensor.matmul(
                            o_ps[m][:], es3[:, :, m * P:(m + 1) * P],
                            vp4[:, r, 2 * tp:2 * tp + 2, 0:DH + 2],
                            start=(u == 0), stop=(u == 15),
                            perf_mode=DR, skip_group_check=True)
                # inline epilogue for this head
                bc_ps = ps_s.tile([P, DH + 2], FP32, tag="st", name=f"bc{h}")
                nc.tensor.matmul(bc_ps[:], ones_row[:],
                                 vrow_r[0:1, h * (DH + 2):(h + 1) * (DH + 2)],
                                 start=True, stop=True, skip_group_check=True)
                bc_sb = sp.tile([P, DH + 2], FP32, tag="bcs", bufs=2, name=f"bcs{h}")
                nc.vector.tensor_copy(bc_sb[:], bc_ps[:])
                for m in range(4):
                    of = sp.tile([P, DH + 2], FP32, tag="of", bufs=2, name=f"of{h}_{m}")
                    nc.vector.tensor_add(of[:], bc_sb[:], o_ps[m][:])
                    recip = sp.tile([P, 1], FP32, tag="rc", bufs=2, name=f"rc{h}_{m}")
                    nc.vector.reciprocal(recip[:], of[:, DH:DH + 1])
                    osc = sp.tile([P, DH], FP32R, tag="osc", bufs=2, name=f"osc{h}_{m}")
                    nc.vector.tensor_scalar_mul(osc[:], of[:, 0:DH], recip[:])
                    for d2 in range(2):
                        tp_ = ps_t.tile([P, P], FP32R, tag="tp")
                        nc.tensor.transpose(tp_[:], osc[:, d2 * P:(d2 + 1) * P], eye[:])
                        nc.vector.tensor_copy(
                            oT[2 * h + d2][:, m * P:(m + 1) * P], tp_[:])

        # ==================================================================
        # phase 4: o-proj + folded-LN FFN + folded-LN trailing stack
        # ==================================================================
        with tc.tile_pool(name="pg2", bufs=6, space="PSUM") as pg2:
            pg2h[0] = pg2
            ln1ctx, ln2ctx = {"src": xRb}, {"src": xR2b}
            q3 = [nc.sync, nc.scalar, nc.gpsimd]

            def oproj_evict(m, pss):
                t = sp.tile([P, TOK], BF16, tag="ev16", bufs=3, name=f"op{m}")
                nc.scalar.activation(t[:], pss[:], AF.Identity,
                                     bias=ob_t[:, m:m + 1])
                nc.vector.tensor_add(xRb[m][:], x3b[m][:], t[:])
                ln_stats(m, ln1ctx, 0)
            gemm_fm(io["ow"], D, D, oT, oproj_evict, pg2, m_outer=True)
            rows1 = ln_rows(ln1ctx, pg2, 0, want_mu_b=True)
            hold1 = {}

            # f1 on the RAW residual (LN folded into f1w' + rank-1 + scale)
            def f1_evict(m, pss):
                tv = sp.tile([P, TOK], FP32, tag="evf", bufs=2, name=f"f1v{m}")
                nc.vector.tensor_mul(tv[:], hold1["rs_b"][:], pss[:])
                nc.scalar.activation(hT[m][:], tv[:], AF.Relu,
                                     bias=f1b_t[:, m:m + 1])
            ln_bcast(rows1, hold1, pg2, 0, True)
            gemm_fm(io["f1w"], D, DFF, xRb, f1_evict, pg2, engines=q3,
                    rank1=(c1f1_r, rows1[0]))
            # lazy-materialize yA = LN1(x) for the f2 residual add
            for k in range(KC):
                t1 = sp.tile([P, TOK], FP32, tag="ev", bufs=2, name=f"lnt1_{k}")
                nc.vector.tensor_sub(t1[:], xRb[k][:], hold1["mu_b"][:])
                t2 = sp.tile([P, TOK], FP32, tag="ev", bufs=2, name=f"lnt2_{k}")
                nc.vector.tensor_mul(t2[:], t1[:], hold1["rs_b"][:])
                nc.scalar.activation(yA[k][:], t2[:], AF.Identity,
                                     scale=ln1g_t[:, k:k + 1],
                                     bias=ln1b_t[:, k:k + 1])

            def f2_evict(m, pss):
                t = sp.tile([P, TOK], BF16, tag="ev16", bufs=3, name=f"f2e{m}")
                nc.scalar.activation(t[:], pss[:], AF.Identity,
                                     bias=f2b_t[:, m:m + 1])
                nc.vector.tensor_add(xR2b[m][:], yA[m][:], t[:])
                ln_stats(m, ln2ctx, 1)
            # f2: two-pass m-outer (8 resident weight tiles per pass) so
            # the eviction + LN2-stats chain streams under pass B.
            f2pss = [pg2.tile([P, TOK], FP32, tag="mm", bufs=8,
                              name=f"psf2_{i}") for i in range(8)]
            for half2 in range(2):
                wts2 = []
                for k in range(8):
                    wt = wp.tile([P, 1024], BF16, tag="w", bufs=12)
                    q3[k % 3].dma_start(
                        wt[:], io["f2w"][(half2 * 8 + k) * P:
                                         (half2 * 8 + k + 1) * P, :])
                    wts2.append(wt)
                for m2 in range(8):
                    for k in range(8):
                        nc.tensor.matmul(
                            f2pss[m2][:], wts2[k][:, m2 * P:(m2 + 1) * P],
                            hT[half2 * 8 + k][:],
                            start=(half2 == 0 and k == 0),
                            stop=(half2 == 1 and k == 7))
                    if half2 == 1:
                        f2_evict(m2, f2pss[m2])
            rows2 = ln_rows(ln2ctx, pg2, 1, want_mu_b=False)
            hold2 = {}

            # fc on the raw second residual (LN2 fully folded)
            def fc_evict(m, pss):
                tv = sp.tile([P, TOK], FP32, tag="evf", bufs=2, name=f"fcv{m}")
                nc.vector.tensor_mul(tv[:], hold2["rs_b"][:], pss[:])
                nc.scalar.activation(yA[m][:], tv[:], AF.Identity,
                                     bias=fcb_t[:, m:m + 1])
            ln_bcast(rows2, hold2, pg2, 1, False)
            gemm_fm(io["fcw"], D, D, xR2b, fc_evict, pg2, engines=q3,
                    rank1=(c1fc_r, rows2[0]))

            def mk_evict(out_tiles, bias_t, relu=False):
                def ev(m, pss):
                    nc.scalar.activation(out_tiles[m][:], pss[:],
                                         AF.Relu if relu else AF.Identity,
                                         bias=bias_t[:, m:m + 1])
                return ev
            gemm_fm(io["k1w"], D, D, yA, mk_evict(yB, k1b_t, relu=True), pg2,
                    engines=q3)

            def out_evict(m, pss):
                fin = sp.tile([P, TOK], FP32, tag="ev", bufs=2, name=f"fin{m}")
                nc.scalar.activation(fin[:], pss[:], AF.Identity,
                                     bias=kob_t[:, m:m + 1])
                q3[m % 3].dma_start(io["outT"][m * P:(m + 1) * P, :], fin[:])
            gemm_fm(io["kow"], D, D, yB, out_evict, pg2, engines=q3,
                    m_outer=True)


def _build():
    nc = bacc.Bacc("TRN2", debug=False, num_devices=NCORES)

    def din(name, shape, dt=BF16):
        return nc.dram_tensor(name, shape, dt, kind="ExternalInput").ap()

    io = {
        "xT": din("xT", [D, TOK]),
        "moew": din("moew", [D, D]),
        "qkw8": din("qkw8", [D, 2 * D], FP8),
        "vw8": din("vw8", [D, D], FP8),
        "vb2048": din("vb2048", [D]),
        "ow": din("ow", [D, D]),
        "f1w": din("f1w", [D, DFF]),
        "f2w": din("f2w", [DFF, D]),
        "fcw": din("fcw", [D, D]),
        "k1w": din("k1w", [D, D]),
        "kow": din("kow", [D, D]),
        "c_onesb": din("c_onesb", [1024], BF16),
        "c_ones": din("c_ones", [256], FP32),
        "c_eye": din("c_eye", [128, 128], FP32),
        "vrow": din("vrow", [H * (DH + 2)], FP32),
        "c1f1": din("c1f1", [DFF], FP32),
        "c1fc": din("c1fc", [D], FP32),
    }
    for name, shape in [("qkb16", [2 * D]), ("ob", [D]), ("f1b", [DFF]),
                        ("f2b", [D]), ("ln1g", [D]), ("ln1b", [D]),
                        ("fcb", [D]), ("k1b", [D]), ("kob", [D]),
                        ("moeb", [D]), ("moeb8", [D])]:
        io[name] = din(name, shape, FP32)
    io["outT"] = nc.dram_tensor("outT", [D, TOK], FP32, kind="ExternalOutput").ap()

    with nc.allow_low_precision("bf16/fp8 matmul pipeline"):
        with tile.TileContext(nc) as tc:
            _body(nc, tc, io)
    nc.compile()
    return nc


# ----------------------------------------------------------------------------
# host side
# ----------------------------------------------------------------------------

def kernel(x, gw, gb, ew, eb, qkvw, qkvb, ow, ob, ln1g, ln1b, ln2g, ln2b,
           f1w, f1b, f2w, f2b, ffw, ffb, cfw, cfb, k1w, k1b, k2w, k2b,
           outw, outb):
    f64 = np.float64
    bf16 = ml_dtypes.bfloat16
    fp8 = ml_dtypes.float8_e4m3
    x = np.asarray(x, np.float32)
    gw, gb = np.asarray(gw, np.float32), np.asarray(gb, np.float32)
    ew, eb = np.asarray(ew, np.float32), np.asarray(eb, np.float32)
    qkvw, qkvb = np.asarray(qkvw, np.float32), np.asarray(qkvb, np.float32)

    # degenerate routing (token 0) + MoE layer fusion, all in f64
    x0 = x[0].astype(f64)
    Ws, bs = [], []
    for l in range(L):
        s = x0 @ gw[l].astype(f64) + gb[l].astype(f64)
        sel = np.argsort(-s, kind="stable")[:2]
        W = (ew[l][sel[0]].astype(f64) + ew[l][sel[1]].astype(f64)) * 0.5
        b = (eb[l][sel[0]].astype(f64) + eb[l][sel[1]].astype(f64)) * 0.5
        x0 = x0 @ W + b
        Ws.append(W)
        bs.append(b)
    Wf = Ws[0] @ Ws[1] @ Ws[2]
    bf_ = bs[0] @ Ws[1] @ Ws[2] + bs[1] @ Ws[2] + bs[2]

    # exact column sums of v for the attention uniform part
    vw_ = qkvw[:, 2 * D:].astype(f64)
    vb_ = qkvb[2 * D:].astype(f64)
    colx3 = x.astype(f64).sum(0) @ Wf + N * bf_
    vsum = colx3 @ vw_ + N * vb_                       # [D]
    vrow = np.zeros((H, DH + 2), np.float32)
    for h in range(H):
        vrow[h, :DH] = (2048.0 * vsum[h * DH:(h + 1) * DH]).astype(np.float32)
        vrow[h, DH] = 128.0 * N
    # LN-folded weights: f1w' = diag(ln1g) @ f1w, etc.
    ln1g64 = np.asarray(ln1g, f64)
    ln2g64 = np.asarray(ln2g, f64)
    f1wp = ln1g64[:, None] * np.asarray(f1w, f64)
    f1bp = np.asarray(ln1b, f64) @ np.asarray(f1w, f64) + np.asarray(f1b, f64)
    c1f1 = -f1wp.sum(0)
    Wfc = np.asarray(ffw, f64) @ np.asarray(cfw, f64)
    bfc = np.asarray(ffb, f64) @ np.asarray(cfw, f64) + np.asarray(cfb, f64)
    fcwp = ln2g64[:, None] * Wfc
    fcbp = np.asarray(ln2b, f64) @ Wfc + bfc
    c1fc = -fcwp.sum(0)
    Wko = np.asarray(k2w, f64) @ np.asarray(outw, f64)
    bko = np.asarray(k2b, f64) @ np.asarray(outw, f64) + np.asarray(outb, f64)

    if "nc" not in _CACHE:
        _CACHE["nc"] = _build()
    nc = _CACHE["nc"]

    shared = {
        "moew": Wf.astype(bf16), "moeb": bf_.astype(np.float32),
        "moeb8": (bf_ * 8.0).astype(np.float32),
        "qkw8": np.clip(np.ascontiguousarray(qkvw[:, :2 * D]) * 256.0,
                        -240, 240).astype(fp8),
        "qkb16": (qkvb[:2 * D] * 16.0).astype(np.float32),
        "vw8": np.clip(np.ascontiguousarray(vw_) * 256.0, -240, 240).astype(fp8),
        "vb2048": (vb_ * 2048.0).astype(bf16),
        "ow": (np.asarray(ow, np.float32) / 16.0).astype(bf16),
        "ob": np.asarray(ob, np.float32),
        "f1w": f1wp.astype(bf16),
        "f1b": f1bp.astype(np.float32),
        "f2w": np.asarray(f2w, np.float32).astype(bf16),
        "f2b": np.asarray(f2b, np.float32),
        "ln1g": np.asarray(ln1g, np.float32), "ln1b": np.asarray(ln1b, np.float32),
        "fcw": fcwp.astype(bf16), "fcb": fcbp.astype(np.float32),
        "k1w": np.asarray(k1w, np.float32).astype(bf16),
        "k1b": np.asarray(k1b, np.float32),
        "kow": Wko.astype(bf16), "kob": bko.astype(np.float32),
        "c_onesb": np.ones(1024, bf16),
        "c_ones": np.ones(256, np.float32),
        "c_eye": np.eye(128, dtype=np.float32),
        "vrow": vrow.reshape(-1),
        "c1f1": c1f1.astype(np.float32),
        "c1fc": c1fc.astype(np.float32),
    }

    in_maps = []
    for c in range(NCORES):
        m = dict(shared)
        m["xT"] = np.ascontiguousarray(x[c * TOK:(c + 1) * TOK].T).astype(bf16)
        in_maps.append(m)

    _CACHE["in_maps"] = in_maps
    res = bass_utils.run_bass_kernel_spmd(nc, in_maps, core_ids=list(range(NCORES)))
    _CACHE["last_result"] = res

    out = np.empty((N, D), np.float32)
    for c in range(NCORES):
        out[c * TOK:(c + 1) * TOK, :] = res.results[c]["outT"].T
    return out
